# revision 15
# baseline (speedup 1.0000x reference)
"""DepthNet cost-volume kernel for 8 Trainium2 NeuronCores.

Strategy: shard output rows (H=128 -> 16 rows/core, +1 halo row each side).
Key geometric fact (verified vs the reference homographies): the source
sample x-coordinate px is independent of the reference row y, and across
all 64 depth hypotheses px moves < 0.5 px and py < 0.15 px.  Hence one
3x3-pixel source patch per (view, ref-row, x) covers the bilinear corners
of ALL 64 depth samples.  The host precomputes, per sample column, the
patch address (one 768B gather descriptor) and the 9 hat-basis blend
weights w[i,j](x,d) = hat(i-fy)*hat(j-fx), which reproduce the reference
bilinear sample (incl. border zeroing) exactly.  The device then does:
gather patches -> 9-tap weighted blend on DVE (fp16) -> accumulate
sum/sum-of-squares -> variance -> PE transpose into conv layout -> 3x3x3
conv as banded fp16 matmuls -> softmax over depth.  No cross-core comm.
"""
import numpy as np
import concourse.bass as bass
import concourse.tile as tile
from concourse import bacc, mybir
from concourse import library_config

F32 = mybir.dt.float32
F16 = mybir.dt.float16
I16 = mybir.dt.int16
OP = mybir.AluOpType
ACT = mybir.ActivationFunctionType

B, C, H, W, D, V = 1, 32, 128, 160, 64, 5
NCORES = 8
ROWS = H // NCORES          # output rows per core
RH = ROWS + 2               # ref rows incl. one halo row each side
NB = 5                      # row batches: 4+4+4+4+2
BROWS = 4                   # rows per full batch
XCOL = 256                  # table columns per erow (padded power-ish)
XOFF = 2                    # table column offset for src x
NTR = RH * XCOL             # table rows per view
ESZ = 384                   # gather elem f16 (3 table rows of 128)
NQ = 16                     # conv chunks of the (c,d') contraction
_cache = {}


def _batch_info(b):
    nrow = BROWS if b < NB - 1 else RH - BROWS * (NB - 1)
    return nrow, (nrow * W + 127) // 128       # rows, chunks


def _build_program(nrep=1):
    nc = bacc.Bacc("TRN2", target_bir_lowering=False, debug=False,
                   num_devices=NCORES, num_swdge_queues=4)
    tabs = [nc.dram_tensor(f"tab{v}", [NTR, 128], F16, kind="ExternalInput")
            for v in range(1, V)]
    idxh = nc.dram_tensor("idx", [128, (V - 1) * NB * 40], I16,
                          kind="ExternalInput")
    wtsh = nc.dram_tensor("wts", [(V - 1) * NB, 128, 2880], F16,
                          kind="ExternalInput")
    f0h = nc.dram_tensor("f0b", [128, NB * 5 * C], F16, kind="ExternalInput")
    bandh = nc.dram_tensor("band", [9 * NQ, 128, 64], F16,
                           kind="ExternalInput")
    identh = nc.dram_tensor("ident", [128, 128], F32, kind="ExternalInput")
    ident16h = nc.dram_tensor("ident16", [128, 128], F16,
                              kind="ExternalInput")
    outh = nc.dram_tensor("out", [ROWS, W, D], F32, kind="ExternalOutput")

    with tile.TileContext(nc) as tc:
        _emit(tc, nc, tabs, idxh, wtsh, f0h, bandh, identh, ident16h, outh,
              nrep)
    cnt = 0
    for blk in nc.m.functions[0].blocks:
        for inst in blk.instructions:
            if isinstance(inst, mybir.InstDMAGatherAnt):
                inst.queue_num = cnt % 4
                cnt += 1
    nc.compile()
    return nc


def _emit(tc, nc, tabs, idxh, wtsh, f0h, bandh, identh, ident16h, outh, nrep):
    import contextlib
    import os
    stage = os.environ.get("K_STAGE", "all")
    with contextlib.ExitStack() as ctx:
        const_p = ctx.enter_context(tc.tile_pool(name="const", bufs=1))
        libi = nc.gpsimd.load_library(library_config.mlp)
        tc._libi = libi

        ident16 = const_p.tile([128, 128], F16)
        nc.sync.dma_start(ident16[:], ident16h.ap())
        idxt = const_p.tile([128, (V - 1) * NB * 40], I16)
        nc.sync.dma_start(idxt[:], idxh.ap())
        f0t = const_p.tile([128, NB * 5 * C], F16)
        nc.sync.dma_start(f0t[:], f0h.ap())

        halo_p = ctx.enter_context(tc.tile_pool(name="halo", bufs=1))
        halos = []
        for k in range(NQ):
            hk = halo_p.tile([128, RH * 162], F16, tag=f"halo{k}")
            halos.append(hk)

        for rep in range(nrep):
            if rep > 0:
                tc.strict_bb_all_engine_barrier()
            for k in range(NQ):
                nc.vector.memset(
                    halos[k][:].rearrange("p (r x) -> p r x", x=162)
                    [:, :, 0:162:161], 0.0)
            if stage in ("all", "main"):
                _emit_main(ctx, tc, nc, tabs, idxt, wtsh, f0t, ident16,
                           halos, rep)
            if stage == "all":
                tc.strict_bb_all_engine_barrier()
            if stage in ("all", "conv"):
                _emit_conv(ctx, tc, nc, bandh, identh, halos, outh, rep)


def _emit_main(ctx, tc, nc, tabs, idxt, wtsh, f0t, ident16, halos, rep):
    import contextlib
    with contextlib.ExitStack() as st:
        gp = st.enter_context(tc.tile_pool(name="gath", bufs=2))
        wp = st.enter_context(tc.tile_pool(name="wts", bufs=2))
        ap_ = st.enter_context(tc.tile_pool(name="acc", bufs=1))
        tpp = st.enter_context(tc.tile_pool(name="tpsum", bufs=4,
                                            space="PSUM"))

        for b in range(NB):
            nrow, CH = _batch_info(b)
            nds = CH * 128
            vsum = ap_.tile([128, 5 * 2048], F16, tag="vsum")
            vsq = ap_.tile([128, 5 * 2048], F16, tag="vsq")
            wrp = ap_.tile([128, 5 * 2048], F16, tag="wrp")
            sqs = ap_.tile([128, 5 * C], F16, tag="sqs")

            vs_v = (vsum[:, :CH * 2048]
                    .rearrange("p (ch c d) -> p ch c d", c=C, d=D))
            vq_v = (vsq[:, :CH * 2048]
                    .rearrange("p (ch c d) -> p ch c d", c=C, d=D))
            wr_v = (wrp[:, :CH * 2048]
                    .rearrange("p (ch c d) -> p ch c d", c=C, d=D))

            # init from ref view: vsum = f0 (bcast over d), vsq = f0^2
            f0v = (f0t[:, b * 5 * C:b * 5 * C + CH * C]
                   .rearrange("p (ch c) -> p ch c", c=C))
            nc.vector.tensor_copy(
                vs_v, f0v.unsqueeze(3).broadcast_to([128, CH, C, D]))
            sq_v = (sqs[:, :CH * C].rearrange("p (ch c) -> p ch c", c=C))
            nc.scalar.activation(sq_v, f0v, ACT.Square)
            nc.vector.tensor_copy(
                vq_v, sq_v.unsqueeze(3).broadcast_to([128, CH, C, D]))

            for v in range(V - 1):
                wt = wp.tile([128, 2880], F16, tag="wt")
                nc.sync.dma_start(
                    wt[:], bass.AP(wtsh, (v * NB + b) * 128 * 2880,
                                   [[2880, 128], [1, 2880]]))
                g = gp.tile([128, 5 * ESZ], F16, tag="g")
                g_v = (g[:, :CH * ESZ]
                       .rearrange("p (ch e) -> p ch e", e=ESZ))
                tab_ap = bass.AP(tabs[v], 0, [[128, NTR - 2], [1, ESZ]])
                ioff = (v * NB + b) * 40
                gi = nc.gpsimd.dma_gather(
                    g_v, tab_ap, idxt[:, ioff:ioff + (nds // 16)],
                    nds, nds, ESZ, elem_step=128)
                tile.add_dep_helper(gi.ins, tc._libi.ins, sync=False,
                                    reason="gather needs mlp library")

                wt_v = (wt[:, :].rearrange("p (ch m d) -> p ch m d",
                                           m=9, d=D)[:, :CH])
                for m in range(9):
                    i, j = divmod(m, 3)
                    pv = (g_v[:, :, j * 128 + i * C:j * 128 + i * C + C]
                          .unsqueeze(3).broadcast_to([128, CH, C, D]))
                    wv = (wt_v[:, :, m:m + 1, :]
                          .broadcast_to([128, CH, C, D]))
                    if m == 0:
                        nc.vector.tensor_tensor(wr_v, pv, wv, op=OP.mult)
                    else:
                        tmp = ap_.tile([128, 5 * 2048], F16, tag="tmp")
                        tm_v = (tmp[:, :CH * 2048]
                                .rearrange("p (ch c d) -> p ch c d",
                                           c=C, d=D))
                        nc.vector.tensor_tensor(tm_v, pv, wv, op=OP.mult)
                        nc.vector.tensor_tensor(wr_v, wr_v, tm_v, op=OP.add)
                nc.vector.tensor_tensor(vs_v, vs_v, wr_v, op=OP.add)
                nc.scalar.activation(wr_v, wr_v, ACT.Square)
                nc.vector.tensor_tensor(vq_v, vq_v, wr_v, op=OP.add)

            # var = vsq/V - (vsum/V)^2, into vsum tile
            nc.scalar.activation(wr_v, vs_v, ACT.Square, scale=1.0 / V)
            nc.vector.scalar_tensor_tensor(vs_v, vq_v, 1.0 / V, wr_v,
                                           op0=OP.mult, op1=OP.subtract)

            # transpose [desc, (c,d)] -> [(c2,d), desc] and place into halos
            var_f = vsum[:, :CH * 2048]
            for ch in range(CH):
                segs = []
                i0 = (ch * 128)
                while i0 < min((ch + 1) * 128, nrow * W):
                    ly = i0 // W
                    ln = min((ly + 1) * W, (ch + 1) * 128, nrow * W) - i0
                    segs.append((i0 - ch * 128, b * BROWS + ly,
                                 i0 - ly * W, ln))
                    i0 += ln
                for t in range(NQ):
                    pt = tpp.tile([128, 128], F16, tag="pt")
                    nc.tensor.transpose(
                        pt[:], var_f[:, ch * 2048 + t * 128:
                                     ch * 2048 + (t + 1) * 128],
                        ident16[:])
                    hv = halos[t][:].rearrange("p (r x) -> p r x", x=162)
                    for (s, gly, x0, ln) in segs:
                        nc.vector.tensor_copy(
                            hv[:, gly, 1 + x0:1 + x0 + ln],
                            pt[:, s:s + ln])


def _emit_conv(ctx, tc, nc, bandh, identh, halos, outh, rep):
    """3x3x3 conv via banded fp16 matmuls + softmax over depth, per row."""
    import contextlib
    with contextlib.ExitStack() as st:
        cp = st.enter_context(tc.tile_pool(name="cconst", bufs=1))
        pp = st.enter_context(tc.tile_pool(name="cpsum", bufs=2, space="PSUM"))
        sp = st.enter_context(tc.tile_pool(name="soft", bufs=2))

        band = cp.tile([128, 9 * NQ * 64], F16)
        nc.sync.dma_start(
            band[:], bass.AP(bandh, 0, [[64, 128], [8192, 9 * NQ], [1, 64]]))
        ident = cp.tile([128, 128], F32)
        nc.sync.dma_start(ident[:], identh.ap())

        for ro in range(1, ROWS + 1):
            cost = pp.tile([64, W], F32, tag="cost")
            first = True
            for dy in range(3):
                for dx in range(3):
                    t = dy * 3 + dx
                    for k in range(NQ):
                        hv = halos[k][:]
                        rhs = hv[:, (ro + dy - 1) * 162 + dx:
                                 (ro + dy - 1) * 162 + dx + W]
                        lhsT = band[:, (t * NQ + k) * 64:(t * NQ + k + 1) * 64]
                        last = (dy == 2 and dx == 2 and k == NQ - 1)
                        nc.tensor.matmul(cost[:], lhsT, rhs,
                                         start=first, stop=last)
                        first = False
            cs = sp.tile([64, W], F32, tag="cs")
            nc.scalar.copy(cs[:], cost[:])
            for xi, (xa, xb) in enumerate(((0, 128), (128, 160))):
                n = xb - xa
                pt = pp.tile([128, 64], F32, tag="pt")
                nc.tensor.transpose(pt[:n, :], cs[:, xa:xb], ident[0:64, 0:64])
                ct = sp.tile([128, 64], F32, tag="ct")
                nc.vector.tensor_copy(ct[:n, :], pt[:n, :])
                mx = sp.tile([128, 1], F32, tag="mx")
                nc.vector.tensor_reduce(mx[:n, :], ct[:n, :],
                                        axis=mybir.AxisListType.X, op=OP.max)
                nc.vector.tensor_scalar(mx[:n, :], mx[:n, :], -1.0, None,
                                        op0=OP.mult)
                ex = sp.tile([128, 64], F32, tag="ex")
                se = sp.tile([128, 1], F32, tag="se")
                nc.scalar.activation(ex[:n, :], ct[:n, :], ACT.Exp,
                                     bias=mx[:n, :], accum_out=se[:n, :])
                nc.vector.reciprocal(se[:n, :], se[:n, :])
                nc.vector.tensor_scalar(ex[:n, :], ex[:n, :], se[:n, :], None,
                                        op0=OP.mult)
                out_ap = bass.AP(outh, (ro - 1) * W * D + xa * D,
                                 [[D, n], [1, D]])
                nc.sync.dma_start(out_ap, ex[:n, :])


def _get_runner(nrep=1):
    if nrep in _cache:
        return _cache[nrep]
    import jax
    from jax.sharding import Mesh, PartitionSpec
    from jax.experimental.shard_map import shard_map
    from concourse.bass2jax import (_bass_exec_p, install_neuronx_cc_hook,
                                    partition_id_tensor)

    nc = _build_program(nrep)
    install_neuronx_cc_hook()
    partition_name = (nc.partition_id_tensor.name
                      if nc.partition_id_tensor else None)
    in_names, out_names, out_avals, zero_outs = [], [], [], []
    for alloc in nc.m.functions[0].allocations:
        if not isinstance(alloc, mybir.MemoryLocationSet):
            continue
        name = alloc.memorylocations[0].name
        if alloc.kind == "ExternalInput":
            if name != partition_name:
                in_names.append(name)
        elif alloc.kind == "ExternalOutput":
            shape = tuple(alloc.tensor_shape)
            dtype = mybir.dt.np(alloc.dtype)
            out_names.append(name)
            out_avals.append(jax.core.ShapedArray(shape, dtype))
            zero_outs.append(np.zeros(shape, dtype))
    n_params, n_outs = len(in_names), len(out_avals)
    all_in = list(in_names) + list(out_names) + (
        [partition_name] if partition_name else [])

    def _body(*args):
        operands = list(args)
        if partition_name is not None:
            operands.append(partition_id_tensor())
        outs = _bass_exec_p.bind(
            *operands, out_avals=tuple(out_avals), in_names=tuple(all_in),
            out_names=tuple(out_names), lowering_input_output_aliases=(),
            sim_require_finite=True, sim_require_nnan=True, nc=nc)
        return tuple(outs)

    devices = jax.devices()[:NCORES]
    mesh = Mesh(np.asarray(devices), ("core",))
    in_specs = (PartitionSpec("core"),) * (n_params + n_outs)
    out_specs = (PartitionSpec("core"),) * n_outs
    donate = tuple(range(n_params, n_params + n_outs))
    sharded = jax.jit(
        shard_map(_body, mesh=mesh, in_specs=in_specs, out_specs=out_specs,
                  check_rep=False),
        donate_argnums=donate, keep_unused=True)

    def run(in_maps):
        per_core = [[np.asarray(m[n]) for n in in_names] for m in in_maps]
        concat_in = [
            np.concatenate([per_core[c][i] for c in range(NCORES)], axis=0)
            for i in range(n_params)]
        concat_zeros = [
            np.zeros((NCORES * z.shape[0], *z.shape[1:]), z.dtype)
            for z in zero_outs]
        out_arrs = sharded(*concat_in, *concat_zeros)
        jax.block_until_ready(out_arrs)
        return [{n: np.asarray(out_arrs[i]).reshape(
                    NCORES, *out_avals[i].shape)[c]
                 for i, n in enumerate(out_names)} for c in range(NCORES)]

    _cache[nrep] = run
    return run


def _host_prep(feat0, feat1, feat2, feat3, feat4, proj_matrices, depth_values,
               conv_w):
    feats = [np.asarray(f, np.float32)[0] for f in
             (feat0, feat1, feat2, feat3, feat4)]          # [C,H,W]
    projs = np.asarray(proj_matrices, np.float64)[0]        # [V,2,4,4]
    depth = np.asarray(depth_values, np.float64)[0]         # [D]
    w3 = np.asarray(conv_w, np.float32)[0]                  # [C,3,3,3]

    def fuse(p):
        out = p[0].copy()
        out[:3, :4] = p[1, :3, :3] @ p[0, :3, :4]
        return out

    ref_inv = np.linalg.inv(fuse(projs[0]))
    xx = np.arange(W, dtype=np.float64)

    # per view: px/py structure (px independent of ref row y)
    Rts, X0s = [], []
    for v in range(1, V):
        P = fuse(projs[v]) @ ref_inv
        R, t = P[:3, :3], P[:3, 3]
        num = R[:, :1] * xx[None, :] + R[:, 2:3]            # [3,W] (no y term)
        p0 = num[:, None, :] * depth[None, :, None] + t[:, None, None]
        # y contributes R[:,1]*y*depth to the numerator
        px = (p0[0]) / (p0[2])                               # [D,W]; y-free since R01=R21=0
        assert abs(R[2, 1]) < 1e-12 and abs(R[0, 1]) < 1e-12
        X0 = np.floor(px.min(axis=0)).astype(np.int64)       # [W]
        fxr = px - X0[None, :]
        assert fxr.min() >= 0 and fxr.max() < 2, (fxr.min(), fxr.max())
        assert X0.min() + XOFF >= 0 and X0.max() + XOFF + 2 < XCOL, (
            X0.min(), X0.max())
        Rts.append((R, t))
        X0s.append(X0)

    band = np.zeros((9, NQ, 128, 64), np.float32)
    d_ = np.arange(64)
    dz = d_[:, None] - d_[None, :] + 1
    msk = (dz >= 0) & (dz < 3)
    dzc = np.clip(dz, 0, 2)
    for dy in range(3):
        for dx in range(3):
            for k in range(NQ):
                for cl in range(2):
                    c = 2 * k + cl
                    blk = np.where(msk, w3[c, dzc, dy, dx], 0.0)
                    band[dy * 3 + dx, k, cl * 64:(cl + 1) * 64, :] = blk
    band = band.reshape(9 * NQ, 128, 64).astype(np.float16)
    ident = np.eye(128, dtype=np.float32)

    in_maps = []
    for core in range(NCORES):
        base = core * ROWS
        m = {"band": band, "ident": ident,
             "ident16": ident.astype(np.float16)}
        # tables: erow in [0,RH) maps to ref row y=base-1+erow; content rows
        # y-1..y+1 zero-padded outside the image; col xcol = x_src + XOFF.
        for v in range(1, V):
            tab = np.zeros((RH, XCOL, 128), np.float16)
            fpad = np.zeros((H + 4, W, C), np.float16)       # global row g -> fpad[g+2]
            fpad[2:H + 2] = feats[v].transpose(1, 2, 0)
            for erow in range(RH):
                y = base - 1 + erow
                tri = fpad[y + 1:y + 4]                      # [3,W,C] rows y-1..y+1
                tab[erow, XOFF:XOFF + W, :96] = (
                    tri.transpose(1, 0, 2).reshape(W, 96))
            m[f"tab{v}"] = tab.reshape(NTR, 128)

        idx = np.zeros(((V - 1), NB, 128, 40), np.int16)
        wts = np.zeros(((V - 1) * NB, 128, 2880), np.float16)
        f0b = np.zeros((128, NB * 5 * C), np.float16)
        f0pad = np.zeros((H + 4, C, W), np.float32)          # global row g -> f0pad[g+2]
        f0pad[2:H + 2] = feats[0].transpose(1, 0, 2)

        for b in range(NB):
            nrow, CH = _batch_info(b)
            nds = nrow * W
            lys = np.arange(nds) // W                        # local row in batch
            xs = np.arange(nds) % W
            ly = b * BROWS + lys                             # row in [0,RH)
            yg = base - 1 + ly                               # global ref row
            # f0 in blend layout [p, ch, c]
            f0v = f0pad[yg + 2][np.arange(nds), :, xs]       # [nds, C] (0 if halo)
            f0v[(yg < 0) | (yg >= H)] = 0.0
            fb = np.zeros((5 * 128, C), np.float32)
            fb[:nds] = f0v
            f0b[:, b * 5 * C:(b + 1) * 5 * C] = (
                fb.reshape(5, 128, C).transpose(1, 0, 2).reshape(128, 5 * C)
                .astype(np.float16))
            for v in range(1, V):
                R, t = Rts[v - 1]
                X0 = X0s[v - 1]
                e = (ly * XCOL + X0[xs] + XOFF).astype(np.int16)  # [nds]
                epad = np.zeros(640, np.int16)
                epad[:nds] = e
                # wrapped idx layout: value[p, col] = e[col*16 + p%16]
                iv = epad.reshape(40, 16).T                  # [16, 40]
                idx[v - 1, b] = np.tile(iv, (8, 1))
                # weights [nds, 9, D]
                valid = (yg >= 0) & (yg < H)
                numx = (R[0, 0] * xs + R[0, 2])[:, None] * depth[None, :] + t[0]
                numy = ((R[1, 0] * xs + R[1, 2] + R[1, 1] * yg)[:, None]
                        * depth[None, :] + t[1])
                den = (R[2, 0] * xs + R[2, 2])[:, None] * depth[None, :] + t[2]
                px = numx / den
                py = numy / den
                fx = px - X0[xs][:, None]                    # [nds, D] in [0,2)
                fy = py - yg[:, None]                        # [nds, D] in (-1,1)
                if valid.any():
                    fv = fy[valid]
                    assert fv.min() > -1 and fv.max() < 1, (
                        fv.min(), fv.max())
                wj = np.stack([np.maximum(0.0, 1 - np.abs(j - fx))
                               for j in range(3)])           # [3,nds,D]
                wi = np.stack([np.maximum(0.0, 1 - np.abs(i - fy))
                               for i in (-1, 0, 1)])          # [3,nds,D]
                wm = (wi[:, None] * wj[None]).reshape(9, nds, D)
                wm = wm * valid[None, :, None]
                wfull = np.zeros((9, 640, D), np.float32)
                wfull[:, :nds] = wm
                # layout [p, ch, m, d]
                wts[(v - 1) * NB + b] = (
                    wfull.reshape(9, 5, 128, D).transpose(2, 1, 0, 3)
                    .reshape(128, 2880).astype(np.float16))
        m["idx"] = idx.transpose(2, 0, 1, 3).reshape(128, (V - 1) * NB * 40)
        m["wts"] = wts
        m["f0b"] = f0b
        in_maps.append(m)
    return in_maps


def kernel(feat0, feat1, feat2, feat3, feat4, proj_matrices, depth_values,
           num_depth=None, conv_w=None, conv_b=None, **_):
    in_maps = _host_prep(feat0, feat1, feat2, feat3, feat4, proj_matrices,
                         depth_values, conv_w)
    run = _get_runner(1)
    res = run(in_maps)
    out = np.zeros((B, D, H, W), np.float32)
    for core in range(NCORES):
        o = res[core]["out"]                                 # [ROWS, W, D]
        out[0, :, core * ROWS:(core + 1) * ROWS, :] = o.transpose(2, 0, 1)
    return out
